# revision 1
# baseline (speedup 1.0000x reference)
"""Trainium2 Bass kernel for a batched attention-like module.

Per batch b:
    a   = sigmoid(z[b] @ M @ e[b]^T)          # [N, N]
    A   = softmax(a, axis=Nz)                 # softmax over the z-row dim
    out = A @ e[b]                            # [N, D]

Strategy (pure data parallel over the batch, 2 batches per NeuronCore, 8 cores):
    Work with the transposed score matrix PT[m, n] = exp(sigmoid(aT)) / cs[m]:
      * softmax denominator cs[m] is a free-axis reduction (ACT accum_out),
      * the final matmul out[n, d] = sum_m PT[m, n] * e[m, d] uses PT directly
        as the stationary (lhsT) operand, e in its natural layout as rhs.
    Matmul inputs use float32r (FP22 truncation): streams 1 col/cycle like
    bf16 when the moving free dim >= 256, ~2^-14 relative error. PT is stored
    bf16 (post-softmax weights; ~2^-9 quantization is well inside the output
    tolerance).

    Phases per batch:
      A0: zmt = (z M)^T        [D, N]   64 matmuls
      A1: PT  = f(e^T . zmt)   [N, N]   256 matmuls + 2 ACT passes/tile
          (sigmoid(x) = 0.5*tanh(0.5*x)+0.5; tanh+exp share one ACT table set)
      B:  out = PT^T . e       [N, D]   256 matmuls
    A1 is ACT-bound (~4.1us/tile ACT vs 3.4us/tile PE), so the first four B
    output groups are split into two half-contractions: the first halves
    (which need only pt tiles 0..7) interleave into A1's second half to keep
    the PE busy while the activation stream paces the psum recycling; their
    second halves absorb the activation drain right after A1's last matmul.

Inputs are transposed on the host (layout prep only; all FLOPs on device).
"""

import sys

sys.path.insert(0, "/opt/trn_rl_repo")

import numpy as np

import concourse.bass as bass
import concourse.tile as tile
from concourse import bacc, mybir

P = 128
F32 = mybir.dt.float32
F32R = mybir.dt.float32r
BF16 = mybir.dt.bfloat16
AF = mybir.ActivationFunctionType

B_FULL, N_FULL, D_FULL = 16, 2048, 512
NCORES = 8


class _Batch:
    """Per-batch emission: pools and tiles with explicit lifetimes.

    SBUF: long-lived tensors (pt, cs, e) on the right side of the heap,
    phase-scoped ones (zmt, ett, stage, zt, ob) LIFO on the left.
    """

    def __init__(self, nc, tc, b, zT, eT, e_nat, out, m_sb, half, dims,
                 uniq=None):
        self.nc, self.tc, self.b = nc, tc, b
        self.uniq = uniq if uniq is not None else str(b)
        self.out = out
        self.m_sb, self.half = m_sb, half
        (self.kd, self.nt, self.nch, self.ch, self.n, self.d) = dims
        self.zT_r = zT[b].rearrange("(kt p) n2 -> p kt n2", p=P)
        self.eT_r = eT[b].rearrange("(kt p) m -> p kt m", p=P)
        self.e_r = e_nat[b].rearrange("(mt p) d2 -> p mt d2", p=P)
        self.etts = {}
        self.stages = {}
        self.post_first_dma = None
        self.n_pref = min(3, self.nt)
        # interleave plan: with nt tiles, split B groups 0..n_early-1 into two
        # half-contractions when the tile count allows it
        self.mid = self.nt // 2
        self.n_early = min(4, self.nt - self.mid) if self.nt >= 8 else 0

    def set_shared(self, ztp, ettp, stp, obp, zmtp, psp):
        self.ztp, self.ettp, self.stp, self.obp = ztp, ettp, stp, obp
        self.zmtp = zmtp
        self.psp = psp

    def open_right(self):
        tc = self.tc
        self.ptp = tc.alloc_tile_pool(name=f"b{self.uniq}_pt", bufs=1, side="right")
        self.csp = tc.alloc_tile_pool(name=f"b{self.uniq}_cs", bufs=1, side="right")
        self.pt = self.ptp.tile([P, self.nt, self.n], BF16, tag="pt")
        self.cs = self.csp.tile([P, self.nt], F32, tag="cs")
        self.recip = self.csp.tile([P, self.nt], F32, tag="recip")

    def load_ett(self, mt):
        t = self.ettp.tile([P, self.kd, P], F32R, tag="ett")
        self.nc.sync.dma_start(out=t, in_=self.eT_r[:, :, mt * P:(mt + 1) * P])
        self.etts[mt] = t

    def a0_chunk(self, c, pool):
        """One n-chunk of zmt[dd, n1] = sum_dp M[dp, dd] * zT[dp, n1].

        psum comes from `pool` (tag ps1): interleaved allocation with A1
        tiles lets these matmuls fill A1's ACT-paced PE gaps.
        """
        nc = self.nc
        kd, ch = self.kd, self.ch
        if c == 0:
            self.zmt = self.zmtp.tile([P, kd, self.n], F32R, tag="zmt")
        zt_ch = self.ztp.tile([P, kd, ch], F32R, tag="zt")
        for k in range(kd):  # split: smaller transfers pipeline better
            nc.sync.dma_start(out=zt_ch[:, k, :],
                              in_=self.zT_r[:, k, c * ch:(c + 1) * ch])
            if k == kd - 1 and self.post_first_dma is not None:
                self.post_first_dma()
                self.post_first_dma = None
        ps = pool.tile([P, self.n], F32, tag="ps1")
        for dt in range(kd):
            sl = slice(dt % (self.n // ch) * ch, dt % (self.n // ch) * ch + ch)
            for k in range(kd):
                nc.tensor.matmul(
                    ps[:, sl],
                    lhsT=self.m_sb[:, k, dt * P:(dt + 1) * P],
                    rhs=zt_ch[:, k, :],
                    start=(k == 0), stop=(k == kd - 1))
            nc.vector.tensor_copy(self.zmt[:, dt, c * ch:(c + 1) * ch],
                                  ps[:, sl])
        if c < self.n_pref:
            self.load_ett(c)  # warm the A1 weight pipeline

    def a0_standalone(self):
        """A0 with nothing to interleave (first batch)."""
        for c in range(self.nch):
            self.a0_chunk(c, self.psp)
        self.finish_a0_prefetch()

    def finish_a0_prefetch(self):
        for mt in range(min(self.nch, self.n_pref), self.n_pref):
            self.load_ett(mt)

    # -- A1 pieces ---------------------------------------------------------
    def _a1_tile_mms(self, mt, pool):
        nc = self.nc
        kd, nch, ch = self.kd, self.nch, self.ch
        if mt + 3 < self.nt:
            self.load_ett(mt + 3)
        # e for phase B in m-tile pieces: no single large transfer blocks
        # the weight loads.
        nc.sync.dma_start(out=self.e_sb[:, mt, :], in_=self.e_r[:, mt, :])
        ett = self.etts.pop(mt)
        ps = pool.tile([P, self.n], F32, tag="ps1")
        for c in range(nch):
            for k in range(kd):
                nc.tensor.matmul(
                    ps[:, c * ch:(c + 1) * ch],
                    lhsT=ett[:, k, :],
                    rhs=self.zmt[:, k, c * ch:(c + 1) * ch],
                    start=(k == 0), stop=(k == kd - 1))
        stage = self.stp.tile([P, self.n], F32, tag="stage")
        nc.scalar.activation(stage, ps, AF.Tanh, scale=0.5)
        self.stages[mt] = stage

    def _act_tail(self, mt):
        """exp + denominator fold for tile mt (runs one tile behind tanh so
        tanh — the PSUM reader — stays ahead in the ACT stream)."""
        nc = self.nc
        stage = self.stages.pop(mt)
        nc.scalar.activation(self.pt[:, mt, :], stage, AF.Exp,
                             bias=self.half[:, 0:1], scale=0.5,
                             accum_out=self.cs[:, mt:mt + 1])
        nc.vector.reciprocal(self.recip[:, mt:mt + 1], self.cs[:, mt:mt + 1])
        nc.vector.tensor_scalar_mul(
            self.pt[:, mt, :], self.pt[:, mt, :], self.recip[:, mt:mt + 1])

    def _b_group_mms(self, ntt, ps, mt_lo, mt_hi, group_start, group_stop):
        nc = self.nc
        for mt in range(mt_lo, mt_hi):
            nc.tensor.matmul(
                ps,
                lhsT=self.pt[:, mt, ntt * P:(ntt + 1) * P],
                rhs=self.e_sb[:, mt, :],
                start=(group_start and mt == mt_lo),
                stop=(group_stop and mt == mt_hi - 1))

    def _b_finalize(self, ntt, ps, obp):
        nc = self.nc
        ob = obp.tile([P, self.d], F32, tag="ob")
        nc.scalar.copy(ob, ps)
        # stores ride the Pool-engine queue so the SP load queue never
        # waits behind them
        nc.gpsimd.dma_start(out=self.out[self.b][ntt * P:(ntt + 1) * P, :],
                            in_=ob)

    def a1(self, nxt=None):
        nc, tc = self.nc, self.tc
        nt, mid = self.nt, self.mid
        self.ep = tc.alloc_tile_pool(name=f"b{self.uniq}_e", bufs=1, side="right")
        self.e_sb = self.ep.tile([P, nt, self.d], BF16, tag="e_sb")
        filler_c = 0
        ps1 = self.psp
        for mt in range(nt):
            self._a1_tile_mms(mt, ps1)
            # exp trails tanh by one tile; the final tanh goes ahead of
            # the last two exps so the PSUM slots (gated on tanh) free
            # as early as possible for the next phase's matmuls.
            if 0 < mt < nt - 1:
                self._act_tail(mt - 1)
            # The ACT stream paces A1; the next batch's A0 matmuls fill the
            # PE surplus (one early, the rest covering the tail), drawing
            # psum from the shared rotation.
            if (nxt is not None and mt in (mid + 2, mid + 4, mid + 6, mid + 7)
                    and filler_c < nxt.nch):
                nxt.a0_chunk(filler_c, ps1)
                filler_c += 1
        if nt > 1:
            self._act_tail(nt - 2)
        self._act_tail(nt - 1)
        if nxt is not None:
            while filler_c < nxt.nch:
                nxt.a0_chunk(filler_c, ps1)
                filler_c += 1
            nxt.finish_a0_prefetch()

    def bphase(self):
        nc = self.nc
        nt = self.nt
        for ntt in range(nt):
            ps_full = self.psp.tile([P, self.n], F32, tag="ps1")
            ps = ps_full[:, :self.d]
            self._b_group_mms(ntt, ps, 0, nt, True, True)
            self._b_finalize(ntt, ps, self.obp)

    def close(self):
        self.ep.release()
        self.csp.release()
        self.ptp.release()


def build(bpc=2, n=N_FULL, d=D_FULL, repeat=1):
    """Build the per-core Bass program (SPMD; same program on all cores).

    Per-core inputs (fp32): zT [bpc, d, n], eT [bpc, d, n], e [bpc, n, d],
    M [d, d].  Output: out [bpc, n, d].
    """
    kd = d // P
    nt = n // P
    nch = max(1, n // 512)
    ch = n // nch
    dims = (kd, nt, nch, ch, n, d)

    nc = bacc.Bacc()
    zT = nc.declare_dram_parameter("zT", [bpc, d, n], F32R, isOutput=False)
    eT = nc.declare_dram_parameter("eT", [bpc, d, n], F32R, isOutput=False)
    e_nat = nc.declare_dram_parameter("e", [bpc, n, d], BF16, isOutput=False)
    M = nc.declare_dram_parameter("M", [d, d], F32R, isOutput=False)
    out = nc.declare_dram_parameter("out", [bpc, n, d], F32, isOutput=True)

    with tile.TileContext(nc) as tc:
        with tc.tile_pool(name="m_pool", bufs=1) as mpool:
            m_sb = mpool.tile([P, kd, d], F32R, tag="m_sb")
            M_r = M.rearrange("(kt p) d2 -> p kt d2", p=P)
            # Only the first k-slice of M is loaded up front; the rest are
            # emitted after the first zT transfer so the first matmul's
            # inputs go through the DMA engines back-to-back.
            nc.sync.dma_start(out=m_sb[:, 0, :], in_=M_r[:, 0, :])

            def _load_m_rest():
                for k in range(1, kd):
                    nc.sync.dma_start(out=m_sb[:, k, :], in_=M_r[:, k, :])

            half = mpool.tile([P, 1], F32, tag="half")
            nc.vector.memset(half, 0.5)

            ztp = tc.alloc_tile_pool(name="sh_zt", bufs=3, side="left")
            ettp = tc.alloc_tile_pool(name="sh_ett", bufs=3, side="left")
            stp = tc.alloc_tile_pool(name="sh_stage", bufs=2, side="left")
            obp = tc.alloc_tile_pool(name="sh_ob", bufs=4, side="left")
            zmtp = tc.alloc_tile_pool(name="sh_zmt", bufs=2, side="left")
            psp = tc.alloc_tile_pool(name="sh_ps", bufs=2, space="PSUM")
            batches = [
                _Batch(nc, tc, b % bpc, zT, eT, e_nat, out, m_sb, half, dims,
                       uniq=str(b))
                for b in range(bpc * repeat)
            ]
            for bt in batches:
                bt.set_shared(ztp, ettp, stp, obp, zmtp, psp)
            batches[0].post_first_dma = _load_m_rest
            # Software pipeline: batch i+1's A0 matmuls interleave into batch
            # i's ACT-paced A1 tail; B(i) follows, then A1(i+1) immediately.
            batches[0].open_right()
            batches[0].a0_standalone()
            for i, bt in enumerate(batches):
                nxt = batches[i + 1] if i + 1 < len(batches) else None
                bt.a1(nxt)
                bt.bphase()
                bt.close()
                if nxt is not None:
                    nxt.open_right()
            for p in (psp, zmtp, obp, stp, ettp, ztp):
                p.release()
    nc.compile()
    return nc


_CACHE = {}


def _get_program():
    if "nc" not in _CACHE:
        _CACHE["nc"] = build()
    return _CACHE["nc"]


def _make_in_maps(z, e, M):
    import ml_dtypes

    z = np.ascontiguousarray(np.asarray(z, dtype=np.float32))
    e = np.ascontiguousarray(np.asarray(e, dtype=np.float32))
    M = np.ascontiguousarray(np.asarray(M, dtype=np.float32))
    zT = np.ascontiguousarray(z.transpose(0, 2, 1))
    eT = np.ascontiguousarray(e.transpose(0, 2, 1))
    # e is only used as the rhs of the final (bf16) matmul; convert on host
    e16 = np.ascontiguousarray(e.astype(ml_dtypes.bfloat16))
    bpc = z.shape[0] // NCORES
    in_maps = []
    for c in range(NCORES):
        sl = slice(c * bpc, (c + 1) * bpc)
        in_maps.append({"zT": zT[sl], "eT": eT[sl], "e": e16[sl], "M": M})
    return in_maps


def run(z, e, M, trace=False):
    """Run on hardware; returns (output [B, N, D], BassKernelResults)."""
    from concourse.bass_utils import run_bass_kernel_spmd

    nc = _get_program()
    in_maps = _make_in_maps(z, e, M)
    res = run_bass_kernel_spmd(nc, in_maps, core_ids=list(range(NCORES)),
                               trace=trace)
    outp = np.concatenate([res.results[c]["out"] for c in range(NCORES)], axis=0)
    return outp, res


def kernel(z, e, M):
    outp, _ = run(z, e, M, trace=False)
    return outp

